# revision 9
# baseline (speedup 1.0000x reference)
"""DisMaxLossFirstPart forward on 8 Trainium2 NeuronCores — fp8 version.

logits = -(iso + mean_c(iso)) / temperature
  iso   = |distance_scale| * sqrt(max(2 - 2*cos(f_b, p_c), 0)) / sqrt(2)
        = sqrt(ds^2 * max(1 - cos(f_b, p_c), 0))

Data-parallel: batch (16384) sharded 8 ways; prototypes replicated; no
collectives (the per-row mean is local).

Host marshalling casts features/prototypes to fp8 e4m3 and lays features
out TRANSPOSED ([F, BS] per core), so the contraction-major operand tiles
(fT) DMA straight from DRAM — no PE transposes and no PSUM->SBUF copies
for features.  All norms are computed on-device from the exact fp8 values.
Main matmuls run in fp8 DoubleRow perf mode (2 contraction chunks per
instruction, 2x bf16 throughput).

Per-core program (B_s = 2048 rows, 16 blocks of 128):
  - fT [128 f, 8 kchunk, 2048 b] fp8 loaded in 16 half-batch DMAs.
  - prototypes: row sumsq on DVE (tensor_tensor_reduce), Sqrt on ACT,
    recip on DVE; normalize FUSED into the PE transpose via
    rhs = diag(-1/||p||) fp8 (built on GPSIMD from an fp8 identity), so
    pT = -(normalized prototypes)^T in one matmul pass; the 16
    PSUM->SBUF fp32->fp8 copies split DVE/ACT.  pT layout
    [128 f, 8 kchunk, 1000 c].
  - per block: row sumsq via 4-matmul DoubleRow Gram + one DVE
    tensor_tensor_reduce against identity (diag) -> Sqrt (ACT) ->
    recip (DVE); mains: per c-chunk (512|488) 4 accumulating DoubleRow
    matmuls into a 2-bank PSUM tile [128,1024]; ONE
    iso = Sqrt(scale_b*psum + ds^2) ACT over [128,1000], bf16 out, with
    accum_out row-sum; m1 = rs/C and logits = (iso + m1)*(-1/T) on
    GPSIMD; output staged 2 blocks per bf16 SBUF tile, 8 output DMAs.
  - host upcasts the bf16 output to fp32.

distance_scale / temperature are [1]-element runtime inputs; their values
are baked into the program as immediates (program rebuilt per call).
"""

import os

import numpy as np

N_CORES = 8
B, F, C = 16384, 1024, 1000
BS = B // N_CORES          # 2048 rows per core
KT = F // 128              # 8 contraction chunks (4 DoubleRow pairs)
CHUNKS = ((0, 512), (512, 488))   # c-chunks of the main matmul
PJ = (C + 127) // 128      # 8 prototype row-tiles (last one 104 rows)


def _build_program(ds2: float, neg_inv_t: float, bs: int = BS):
    from contextlib import ExitStack

    import concourse.tile as tile
    from concourse import bacc, mybir
    from concourse.masks import make_identity

    f32 = mybir.dt.float32
    bf16 = mybir.dt.bfloat16
    fp8 = mybir.dt.float8e4
    AF = mybir.ActivationFunctionType
    ALU = mybir.AluOpType
    DR = mybir.MatmulPerfMode.DoubleRow

    nb = bs // 128
    inv_ds4 = (1.0 / ds2) ** 2 if ds2 != 1.0 else 1.0
    no_dr = bool(int(os.environ.get("KV3_NO_DR", "0")))
    out1 = bool(int(os.environ.get("KV3_OUT1", "0")))

    nc = bacc.Bacc("TRN2", target_bir_lowering=False, debug=False,
                   num_devices=N_CORES)

    fdr = nc.dram_tensor("features_t", [F, bs], fp8,
                         kind="ExternalInput").ap()
    pdr = nc.dram_tensor("prototypes", [C, F], fp8,
                         kind="ExternalInput").ap()
    odr = nc.dram_tensor("out", [bs, C], bf16, kind="ExternalOutput").ap()

    with tile.TileContext(nc) as tc, ExitStack() as ctx:
        const_pool = ctx.enter_context(tc.tile_pool(name="const", bufs=1))
        ident8 = const_pool.tile([128, 128], fp8, tag="ident8")
        make_identity(nc, ident8[:])
        ident32 = const_pool.tile([128, 128], f32, tag="ident32")
        make_identity(nc, ident32[:])
        bias_ds2 = const_pool.tile([128, 1], f32, tag="bias_ds2")
        nc.vector.memset(bias_ds2[:], ds2)
        # warm the ACT table set (Square/Sqrt) during the initial DMA window
        warm = const_pool.tile([128, 1], f32, tag="warm")
        nc.scalar.activation(warm[:], bias_ds2[:], AF.Square)
        nc.scalar.activation(warm[:], warm[:], AF.Sqrt)

        # whole-core transposed features, loaded once
        fT_pool = ctx.enter_context(tc.tile_pool(name="fT", bufs=1))
        fT = fT_pool.tile([128, KT, bs], fp8, tag="fT", name="fT")
        # persistent transposed prototypes
        pT_pool = ctx.enter_context(tc.tile_pool(name="pT", bufs=1))
        pT = pT_pool.tile([128, KT, C], fp8, tag="pT", name="pT")

        # PSUM: spsum banks 0-3; preamble ppsum 4-7, then gram reuses 4-5.
        spsum = ctx.enter_context(tc.tile_pool(name="spsum", bufs=2, space="PSUM"))

        # ---- prototype preamble + feature loads ---------------------------
        with tc.tile_pool(name="pload", bufs=1) as pload, \
             tc.tile_pool(name="ppsum", bufs=4, space="PSUM") as ppsum, \
             tc.tile_pool(name="psmall", bufs=1) as psmall, \
             tc.tile_pool(name="pscr", bufs=2) as pscr:
            praw = []
            for j in range(PJ):
                rows = min(128, C - j * 128)
                pr = pload.tile([128, F], fp8, tag=f"praw{j}", name=f"praw{j}")
                nc.sync.dma_start(out=pr[:rows],
                                  in_=pdr[j * 128: j * 128 + rows])
                praw.append((pr, rows))
            half = bs // 2
            for k in range(KT):
                nc.sync.dma_start(out=fT[:, k, :half],
                                  in_=fdr[k * 128:(k + 1) * 128, :half])
            for k in range(KT):
                nc.sync.dma_start(out=fT[:, k, half:],
                                  in_=fdr[k * 128:(k + 1) * 128, half:])

            Ds = []
            for j in range(PJ):
                pr, rows = praw[j]
                scr = pscr.tile([128, F], f32, tag="pscr")
                ss = psmall.tile([128, 1], f32, tag=f"pss{j}")
                nc.scalar.activation(scr[:rows], pr[:rows], AF.Square,
                                     accum_out=ss[:rows])
                nrm = psmall.tile([128, 1], f32, tag=f"pnrm{j}")
                nc.scalar.activation(nrm[:rows], ss[:rows], AF.Sqrt)
                inv = psmall.tile([128, 1], f32, tag=f"pinv{j}")
                nc.vector.reciprocal(inv[:rows], nrm[:rows])
                # D_j = diag(-1/||p||) in fp8 (negated so psum holds -G)
                Dj = psmall.tile([128, 128], fp8, tag=f"pD{j}")
                nc.gpsimd.tensor_scalar(Dj[:rows, :rows],
                                        ident8[:rows, :rows], inv[:rows],
                                        -1.0, ALU.mult, ALU.mult)
                Ds.append(Dj)

            # transpose+normalize: out[f, c] = praw[c, f] * (-inv[c]).
            # jg-outer so pT[:, :, 0:512] (c-chunk-0 rhs) is ready after
            # only the first 4 prototype tiles.
            rnd = 0

            def p_round(jg, k, j0, nj):
                nonlocal rnd
                cols = sum(praw[j0 + jj][1] for jj in range(nj))
                base = (j0 - jg * 4) * 128
                pt_ps = ppsum.tile([128, 512], f32, tag="ptps",
                                   name=f"ptps_{jg}_{k}_{j0}")
                for jj in range(nj):
                    pr, rows = praw[j0 + jj]
                    nc.tensor.matmul(
                        pt_ps[:, jj * 128: jj * 128 + rows],
                        lhsT=pr[:rows, k * 128:(k + 1) * 128],
                        rhs=Ds[j0 + jj][:rows, :rows], start=True, stop=True)
                dst = pT[:, k, jg * 512 + base: jg * 512 + base + cols]
                if rnd % 8 == 5:
                    nc.scalar.copy(dst, pt_ps[:, :cols])
                else:
                    nc.vector.tensor_copy(dst, pt_ps[:, :cols])
                rnd += 1

            for k in (0, 1):
                p_round(0, k, 0, 2)
                p_round(0, k, 2, 2)
            for k in range(2, KT):
                p_round(0, k, 0, 4)
            for k in range(KT):
                p_round(1, k, 4, 4)

        # gram pool reuses the preamble's PSUM banks (opened after it closes)
        gps = ctx.enter_context(tc.tile_pool(name="gps", bufs=2, space="PSUM"))

        # ---- main loop over feature blocks --------------------------------
        with tc.tile_pool(name="iso", bufs=3) as isop, \
             tc.tile_pool(name="osb", bufs=3) as osbp, \
             tc.tile_pool(name="dscr", bufs=2) as dscrp, \
             tc.tile_pool(name="small", bufs=8) as smallp:

            ob2 = None
            for bi in range(nb):
                b0 = bi * 128
                fTb = fT[:, :, b0:b0 + 128]
                # row sumsq via DoubleRow Gram; diag via tensor_tensor_reduce
                gp = gps.tile([128, 128], f32, tag="gram")
                if no_dr:
                    for k in range(KT):
                        nc.tensor.matmul(gp[:], lhsT=fTb[:, k, :],
                                         rhs=fTb[:, k, :],
                                         start=(k == 0), stop=(k == KT - 1))
                else:
                    for j in range(4):
                        nc.tensor.matmul(gp[:],
                                         lhsT=fTb[:, 2 * j:2 * j + 2, :],
                                         rhs=fTb[:, 2 * j:2 * j + 2, :],
                                         start=(j == 0), stop=(j == 3),
                                         perf_mode=DR)
                dscr = dscrp.tile([128, 128], f32, tag="dscr")
                ss = smallp.tile([128, 1], f32, tag="fss")
                nc.vector.tensor_tensor(dscr[:], gp[:], ident32[:], ALU.mult)
                nc.vector.tensor_reduce(ss[:], dscr[:],
                                        mybir.AxisListType.X, ALU.add)
                nrm = smallp.tile([128, 1], f32, tag="fnrm")
                nc.scalar.activation(nrm[:], ss[:], AF.Sqrt, scale=inv_ds4)
                scl = smallp.tile([128, 1], f32, tag="fscl")
                nc.vector.reciprocal(scl[:], nrm[:])

                sp = spsum.tile([128, 1024], f32, tag="spsum")
                for cbase, cw in CHUNKS:
                    if no_dr:
                        for k in range(KT):
                            nc.tensor.matmul(
                                sp[:, cbase:cbase + cw],
                                lhsT=fTb[:, k, :],
                                rhs=pT[:, k, cbase:cbase + cw],
                                start=(k == 0), stop=(k == KT - 1))
                    else:
                        for j in range(4):
                            nc.tensor.matmul(
                                sp[:, cbase:cbase + cw],
                                lhsT=fTb[:, 2 * j:2 * j + 2, :],
                                rhs=pT[:, 2 * j:2 * j + 2, cbase:cbase + cw],
                                start=(j == 0), stop=(j == 3), perf_mode=DR)

                iso = isop.tile([128, C], bf16, tag="iso")
                rs = smallp.tile([128, 1], f32, tag="rs")
                nc.scalar.activation(iso[:], sp[:, :C], AF.Sqrt,
                                     bias=bias_ds2[:], scale=scl[:],
                                     accum_out=rs[:])
                m1 = smallp.tile([128, 1], f32, tag="m1")
                nc.gpsimd.tensor_scalar(m1[:], rs[:], 1.0 / C, None, ALU.mult)
                if out1:
                    ob = osbp.tile([128, C], bf16, tag="osb")
                    nc.gpsimd.tensor_scalar(ob[:], iso[:], m1[:],
                                            neg_inv_t, ALU.add, ALU.mult)
                    nc.sync.dma_start(out=odr[bi * 128:(bi + 1) * 128],
                                      in_=ob[:])
                else:
                    if bi % 2 == 0:
                        ob2 = osbp.tile([128, 2, C], bf16, tag="osb")
                    nc.gpsimd.tensor_scalar(ob2[:, bi % 2, :], iso[:], m1[:],
                                            neg_inv_t, ALU.add, ALU.mult)
                    if bi % 2 == 1:
                        dst = odr[(bi - 1) * 128:(bi + 1) * 128] \
                            .rearrange("(j p) c -> p j c", j=2)
                        nc.sync.dma_start(out=dst, in_=ob2[:])

    nc.compile()
    return nc


def kernel(features, prototypes, distance_scale, temperature):
    from concourse.bass_utils import run_bass_kernel_spmd

    import ml_dtypes
    f8 = np.ascontiguousarray(features, dtype=np.float32) \
        .astype(ml_dtypes.float8_e4m3)
    p8 = np.ascontiguousarray(prototypes, dtype=np.float32) \
        .astype(ml_dtypes.float8_e4m3)
    ds2 = float(abs(float(np.asarray(distance_scale).reshape(-1)[0])) ** 2)
    neg_inv_t = -1.0 / float(np.asarray(temperature).reshape(-1)[0])

    nc = _build_program(ds2, neg_inv_t)

    in_maps = [{"features_t": np.ascontiguousarray(f8[i * BS:(i + 1) * BS].T),
                "prototypes": p8} for i in range(N_CORES)]

    trace_dir = os.environ.get("KERNEL_TRACE_DIR")
    if trace_dir:
        res = run_bass_kernel_spmd(nc, in_maps, list(range(N_CORES)),
                                   trace=True, tmpdir=trace_dir)
        print(f"HW exec time: {res.exec_time_ns} ns")
        print(f"mean core exec time: {res.mean_exec_time_ns} ns")
    else:
        res = run_bass_kernel_spmd(nc, in_maps, list(range(N_CORES)))

    return np.concatenate(
        [res.results[i]["out"].astype(np.float32) for i in range(N_CORES)],
        axis=0)
